# revision 3
# baseline (speedup 1.0000x reference)
"""BinaryLinear (straight-through sign(w)) kernel for Trainium2, 8 NeuronCores.

Computes out = x @ sign(w).T + b for
  x: [8192, 2048] f32, w: [4096, 2048] f32, b: [4096] f32 -> out [8192, 4096] f32.

Sharding: 4-way data parallel (batch) x 2-way tensor parallel (out_features).
Each core computes a [2048, 2048] block of the output:
  out[bi*2048:(bi+1)*2048, fi*2048:(fi+1)*2048]
    = x_shard @ sign(w_shard).T + b_shard.

Per-core device kernel (bf16 matmul, fp32 accumulate):
  - the whole w^T shard [2048, 2048] bf16 lives in SBUF (64 KiB/partition),
    loaded once;
  - x^T tiles stream through a multi-buffered pool;
  - bias is added during the PSUM->SBUF copyback on the vector engine.
"""

from contextlib import ExitStack

import ml_dtypes
import numpy as np

# Full problem shapes (hardcoded per the grading contract).
M, K, N = 8192, 2048, 4096
P_BATCH, P_FEAT = 4, 2  # 4 x 2 core grid
MC, NC = M // P_BATCH, N // P_FEAT  # 2048, 2048 per-core block
N_CORES = P_BATCH * P_FEAT
P = 128


def build_nc(mc: int = MC, k: int = K, nc_dim: int = NC):
    """Build + compile the per-core Bass module: out[mc, nc_dim] = xt^T @ wt + bias."""
    import concourse.mybir as mybir
    import concourse.tile as tile
    from concourse import bacc
    from concourse.bass import ts
    from concourse.kernels.tile_matmul import (
        ShapeInfo,
        composable_matmul_tile_kernel,
        dma_from_dram_kxm,
        dma_to_dram_mxn,
    )

    ko = k // P
    nc = bacc.Bacc("TRN2", target_bir_lowering=False, debug=False)
    xt = nc.dram_tensor("xt", [k, mc], mybir.dt.bfloat16, kind="ExternalInput")
    wt = nc.dram_tensor("wt", [k, nc_dim], mybir.dt.bfloat16, kind="ExternalInput")
    bias = nc.dram_tensor("bias", [nc_dim], mybir.dt.float32, kind="ExternalInput")
    out = nc.dram_tensor("out", [mc, nc_dim], mybir.dt.float32, kind="ExternalOutput")

    MAX_K_TILE = 512
    k_tile = min(MAX_K_TILE, k)
    k_tiles = k // k_tile
    k_subtiles = k_tile // P

    with tile.TileContext(nc) as tc, ExitStack() as ctx:
        const = ctx.enter_context(tc.tile_pool(name="const", bufs=1))
        kxm_pool = ctx.enter_context(tc.tile_pool(name="kxm", bufs=k_tiles + 1))

        # Bias replicated across all 128 partitions so the copyback can add the
        # n-slice with a plain tensor_tensor add.
        bias_sb = const.tile([P, nc_dim], mybir.dt.float32)
        nc.sync.dma_start(
            out=bias_sb[:], in_=bias.ap()[None, :].to_broadcast((P, nc_dim))
        )

        # Whole w^T shard resident in SBUF, laid out [p, ko, n] with
        # cache[p, o, n] = wt[o*128 + p, n]. Loaded in k_tile-sized chunks so
        # the first matmuls don't wait on the full preload.
        w_sb = const.tile([P, ko, nc_dim], mybir.dt.bfloat16)
        wt_t = wt.ap().rearrange("(o p) n -> p o n", p=P)
        for kt in range(k_tiles):
            sl = slice(kt * k_subtiles, (kt + 1) * k_subtiles)
            nc.sync.dma_start(out=w_sb[:, sl, :], in_=wt_t[:, sl, :])

        kxm_producer, kxm_shape = dma_from_dram_kxm(kxm_pool, xt.ap())

        def kxn_producer(nc_, md):
            return w_sb[:, ts(md.k_tile_idx, md.k_subtiles), ts(md.n_tile_idx, md.n_tile)]

        kxn_shape = ShapeInfo(pdims=((P, ko),), fdims=(nc_dim,))

        def add_bias_reducer(nc_, psum, sbuf, md):
            sz = md.n_subtile_slice_size
            nc_.vector.tensor_add(
                out=sbuf[:, :, :sz],
                in0=psum[:, :sz],
                in1=bias_sb[: psum.shape[0], md.n_subtile_slice],
            )

        composable_matmul_tile_kernel(
            tc=tc,
            kxm_shape=kxm_shape,
            kxn_shape=kxn_shape,
            output_type=mybir.dt.float32,
            kxm_producer=kxm_producer,
            kxn_producer=kxn_producer,
            mxn_consumer=dma_to_dram_mxn(out.ap()),
            mxn_subtile_reducer=add_bias_reducer,
            MAX_K_TILE_SIZE=MAX_K_TILE,
            psum_n_bufs=2,
        )

    nc.compile()
    return nc


_NC_CACHE = None


def _get_nc():
    global _NC_CACHE
    if _NC_CACHE is None:
        _NC_CACHE = build_nc()
    return _NC_CACHE


def kernel(x: np.ndarray, w: np.ndarray, b: np.ndarray) -> np.ndarray:
    from concourse.bass_utils import run_bass_kernel_spmd

    x = np.asarray(x, dtype=np.float32)
    w = np.asarray(w, dtype=np.float32)
    b = np.asarray(b, dtype=np.float32)

    bf16 = ml_dtypes.bfloat16
    x_bf = x.astype(bf16)
    w_bf = np.sign(w).astype(bf16)

    # Unique transposed shards (x^T per batch group, sign(w)^T per feature group).
    xt_shards = [
        np.ascontiguousarray(x_bf[bi * MC : (bi + 1) * MC, :].T) for bi in range(P_BATCH)
    ]
    wt_shards = [
        np.ascontiguousarray(w_bf[fi * NC : (fi + 1) * NC, :].T) for fi in range(P_FEAT)
    ]
    b_shards = [np.ascontiguousarray(b[fi * NC : (fi + 1) * NC]) for fi in range(P_FEAT)]

    in_maps = []
    for c in range(N_CORES):
        bi, fi = divmod(c, P_FEAT)
        in_maps.append(
            {"xt": xt_shards[bi], "wt": wt_shards[fi], "bias": b_shards[fi]}
        )

    nc = _get_nc()
    results = run_bass_kernel_spmd(nc, in_maps, core_ids=list(range(N_CORES))).results

    out = np.empty((M, N), dtype=np.float32)
    for c in range(N_CORES):
        bi, fi = divmod(c, P_FEAT)
        out[bi * MC : (bi + 1) * MC, fi * NC : (fi + 1) * NC] = results[c]["out"]
    return out


# revision 5
# speedup vs baseline: 4.7709x; 4.7709x over previous
"""BinaryLinear (straight-through sign(w)) kernel for Trainium2, 8 NeuronCores.

Computes out = x @ sign(w).T + b for
  x: [8192, 2048] f32, w: [4096, 2048] f32, b: [4096] f32 -> out [8192, 4096] f32.

Sharding: 4-way data parallel (batch) x 2-way tensor parallel (out_features).
Each core computes a [2048, 2048] block of the output:
  out[bi*2048:(bi+1)*2048, fi*2048:(fi+1)*2048]
    = x_shard @ sign(w_shard).T + b_shard.

Per-core device kernel (bf16 matmul, fp32 accumulate):
  - the whole w^T shard [2048, 2048] bf16 lives in SBUF (64 KiB/partition),
    loaded once;
  - x^T tiles stream through a multi-buffered pool;
  - bias is added during the PSUM->SBUF copyback on the vector engine.
"""

from contextlib import ExitStack

import ml_dtypes
import numpy as np

# Full problem shapes (hardcoded per the grading contract).
M, K, N = 8192, 2048, 4096
P_BATCH, P_FEAT = 4, 2  # 4 x 2 core grid
MC, NC = M // P_BATCH, N // P_FEAT  # 2048, 2048 per-core block
N_CORES = P_BATCH * P_FEAT
P = 128


def build_nc(mc: int = MC, k: int = K, nc_dim: int = NC, reps: int = 1):
    """Build + compile the per-core Bass module: out[mc, nc_dim] = xt^T @ wt + bias.

    reps > 1 repeats the whole computation (for slope-based benchmarking)."""
    import concourse.mybir as mybir
    import concourse.tile as tile
    from concourse import bacc
    from concourse.bass import ts
    from concourse.kernels.tile_matmul import (
        ShapeInfo,
        composable_matmul_tile_kernel,
        dma_from_dram_kxm,
        dma_to_dram_mxn,
    )

    ko = k // P
    nc = bacc.Bacc("TRN2", target_bir_lowering=False, debug=False)
    xt = nc.dram_tensor("xt", [k, mc], mybir.dt.bfloat16, kind="ExternalInput")
    wt = nc.dram_tensor("wt", [k, nc_dim], mybir.dt.bfloat16, kind="ExternalInput")
    bias = nc.dram_tensor("bias", [nc_dim], mybir.dt.float32, kind="ExternalInput")
    out = nc.dram_tensor("out", [mc, nc_dim], mybir.dt.float32, kind="ExternalOutput")

    MAX_K_TILE = 512
    k_tile = min(MAX_K_TILE, k)
    k_tiles = k // k_tile
    k_subtiles = k_tile // P

    with tile.TileContext(nc) as tc, ExitStack() as ctx:
        const = ctx.enter_context(tc.tile_pool(name="const", bufs=1))
        kxm_pool = ctx.enter_context(tc.tile_pool(name="kxm", bufs=k_tiles + 1))

        # Bias replicated across all 128 partitions so the copyback can add the
        # n-slice with a plain tensor_tensor add.
        bias_sb = const.tile([P, nc_dim], mybir.dt.float32)
        nc.sync.dma_start(
            out=bias_sb[:], in_=bias.ap()[None, :].to_broadcast((P, nc_dim))
        )

        # Whole w^T shard resident in SBUF, laid out [p, ko, n] with
        # cache[p, o, n] = wt[o*128 + p, n]. Loaded in k_tile-sized chunks so
        # the first matmuls don't wait on the full preload.
        w_sb = const.tile([P, ko, nc_dim], mybir.dt.bfloat16)
        wt_t = wt.ap().rearrange("(o p) n -> p o n", p=P)
        for kt in range(k_tiles):
            sl = slice(kt * k_subtiles, (kt + 1) * k_subtiles)
            nc.sync.dma_start(out=w_sb[:, sl, :], in_=wt_t[:, sl, :])

        kxm_producer, kxm_shape = dma_from_dram_kxm(kxm_pool, xt.ap())

        def kxn_producer(nc_, md):
            return w_sb[:, ts(md.k_tile_idx, md.k_subtiles), ts(md.n_tile_idx, md.n_tile)]

        kxn_shape = ShapeInfo(pdims=((P, ko),), fdims=(nc_dim,))

        def add_bias_reducer(nc_, psum, sbuf, md):
            sz = md.n_subtile_slice_size
            nc_.vector.tensor_add(
                out=sbuf[:, :, :sz],
                in0=psum[:, :sz],
                in1=bias_sb[: psum.shape[0], md.n_subtile_slice],
            )

        for _ in range(reps):
            composable_matmul_tile_kernel(
                tc=tc,
                kxm_shape=kxm_shape,
                kxn_shape=kxn_shape,
                output_type=mybir.dt.float32,
                kxm_producer=kxm_producer,
                kxn_producer=kxn_producer,
                mxn_consumer=dma_to_dram_mxn(out.ap()),
                mxn_subtile_reducer=add_bias_reducer,
                MAX_K_TILE_SIZE=MAX_K_TILE,
                psum_n_bufs=2,
            )

    nc.compile()
    return nc


_NC_CACHE = None


def _get_nc():
    global _NC_CACHE
    if _NC_CACHE is None:
        _NC_CACHE = build_nc()
    return _NC_CACHE


def kernel(x: np.ndarray, w: np.ndarray, b: np.ndarray) -> np.ndarray:
    from concourse.bass_utils import run_bass_kernel_spmd

    x = np.asarray(x, dtype=np.float32)
    w = np.asarray(w, dtype=np.float32)
    b = np.asarray(b, dtype=np.float32)

    bf16 = ml_dtypes.bfloat16
    x_bf = x.astype(bf16)
    w_bf = np.sign(w).astype(bf16)

    # Unique transposed shards (x^T per batch group, sign(w)^T per feature group).
    xt_shards = [
        np.ascontiguousarray(x_bf[bi * MC : (bi + 1) * MC, :].T) for bi in range(P_BATCH)
    ]
    wt_shards = [
        np.ascontiguousarray(w_bf[fi * NC : (fi + 1) * NC, :].T) for fi in range(P_FEAT)
    ]
    b_shards = [np.ascontiguousarray(b[fi * NC : (fi + 1) * NC]) for fi in range(P_FEAT)]

    in_maps = []
    for c in range(N_CORES):
        bi, fi = divmod(c, P_FEAT)
        in_maps.append(
            {"xt": xt_shards[bi], "wt": wt_shards[fi], "bias": b_shards[fi]}
        )

    nc = _get_nc()
    results = run_bass_kernel_spmd(nc, in_maps, core_ids=list(range(N_CORES))).results

    out = np.empty((M, N), dtype=np.float32)
    for c in range(N_CORES):
        bi, fi = divmod(c, P_FEAT)
        out[bi * MC : (bi + 1) * MC, fi * NC : (fi + 1) * NC] = results[c]["out"]
    return out


# revision 12
# speedup vs baseline: 5.0376x; 1.0559x over previous
"""BinaryLinear (straight-through sign(w)) kernel for Trainium2, 8 NeuronCores.

Computes out = x @ sign(w).T + b for
  x: [8192, 2048] f32, w: [4096, 2048] f32, b: [4096] f32 -> out [8192, 4096] f32.

Sharding: 4-way data parallel (batch) x 2-way tensor parallel (out_features).
Each core computes a [2048, 2048] block of the output:
  out[bi*2048:(bi+1)*2048, fi*2048:(fi+1)*2048]
    = x_shard @ sign(w_shard).T + b_shard.

Per-core device kernel (bf16 matmul, fp32 accumulate):
  - the whole w^T shard [2048, 2048] bf16 lives in SBUF (64 KiB/partition),
    loaded once;
  - x^T tiles stream through a multi-buffered pool;
  - bias is added during the PSUM->SBUF copyback on the vector engine.
"""

from contextlib import ExitStack

import ml_dtypes
import numpy as np

# Full problem shapes (hardcoded per the grading contract).
M, K, N = 8192, 2048, 4096
P_BATCH, P_FEAT = 4, 2  # 4 x 2 core grid
MC, NC = M // P_BATCH, N // P_FEAT  # 2048, 2048 per-core block
N_CORES = P_BATCH * P_FEAT
P = 128


def build_nc(mc: int = MC, k: int = K, nc_dim: int = NC, reps: int = 1):
    """Build + compile the per-core Bass module: out[mc, nc_dim] = xt^T @ wt + bias.

    reps > 1 repeats the whole computation (for slope-based benchmarking)."""
    import concourse.mybir as mybir
    import concourse.tile as tile
    from concourse import bacc
    from concourse.bass import ts
    from concourse.kernels.tile_matmul import (
        ShapeInfo,
        composable_matmul_tile_kernel,
    )

    ko = k // P
    nc = bacc.Bacc("TRN2", target_bir_lowering=False, debug=False)
    xt = nc.dram_tensor("xt", [k, mc], mybir.dt.bfloat16, kind="ExternalInput")
    wt = nc.dram_tensor("wt", [k, nc_dim], mybir.dt.bfloat16, kind="ExternalInput")
    bias = nc.dram_tensor("bias", [nc_dim], mybir.dt.float32, kind="ExternalInput")
    out = nc.dram_tensor("out", [mc, nc_dim], mybir.dt.float32, kind="ExternalOutput")

    MAX_K_TILE = 512
    k_tile = min(MAX_K_TILE, k)
    k_tiles = k // k_tile
    k_subtiles = k_tile // P

    with tile.TileContext(nc) as tc, ExitStack() as ctx:
        const = ctx.enter_context(tc.tile_pool(name="const", bufs=1))
        kxm_pool = ctx.enter_context(tc.tile_pool(name="kxm", bufs=k_tiles + 1))

        # Whole w^T shard resident in SBUF, laid out [p, ko, n] with
        # cache[p, o, n] = wt[o*128 + p, n]. Preload runs on the gpsimd (SWDGE)
        # queue so the x-tile loads (HWDGE via nc.sync) are not serialized
        # behind it, and in fine chunks ordered so the chunk the first matmuls
        # need lands first.
        w_sb = const.tile([P, ko, nc_dim], mybir.dt.bfloat16)
        wt_t = wt.ap().rearrange("(o p) n -> p o n", p=P)
        # n-major order: the first output tile consumes (n0, k0..k3), so all
        # its chunks must land first. The very first chunk is split per
        # k-subtile so the first matmul unblocks after ~128 KiB.
        n_chunk = max(512, nc_dim // 4)
        for n0 in range(0, nc_dim, n_chunk):
            for kt in range(k_tiles):
                sl = slice(kt * k_subtiles, (kt + 1) * k_subtiles)
                if n0 == 0 and kt == 0:
                    for s in range(k_subtiles):
                        nc.gpsimd.dma_start(
                            out=w_sb[:, s : s + 1, n0 : n0 + n_chunk],
                            in_=wt_t[:, s : s + 1, n0 : n0 + n_chunk],
                        )
                else:
                    nc.gpsimd.dma_start(
                        out=w_sb[:, sl, n0 : n0 + n_chunk],
                        in_=wt_t[:, sl, n0 : n0 + n_chunk],
                    )

        # Bias replicated across all 128 partitions so the copyback can add the
        # n-slice with a plain tensor_tensor add. First needed only at the
        # first PSUM eviction, so it goes after the first w chunks.
        bias_sb = const.tile([P, nc_dim], mybir.dt.float32)
        nc.gpsimd.dma_start(
            out=bias_sb[:], in_=bias.ap()[None, :].to_broadcast((P, nc_dim))
        )

        # Custom kxm producer: one DMA per k-subtile (instead of one per
        # k-tile) so the first matmuls unblock sooner and later tiles
        # prefetch at finer granularity.
        xt_t = xt.ap().rearrange("(o p) m -> p o m", p=P)

        def kxm_producer(nc_, md):
            t = kxm_pool.tile([P, md.k_subtiles, md.m_tile], mybir.dt.bfloat16, tag="kxm")
            m0 = md.m_tile_idx * md.m_tile
            o0 = md.k_tile_idx * md.k_subtiles
            if md.k_tile_idx == 0 and md.m_tile_idx == 0:
                # Fine-grained only on the critical first tile so the first
                # matmul unblocks after one k-subtile instead of four.
                for s in range(md.k_subtiles):
                    nc_.sync.dma_start(
                        out=t[:, s, :], in_=xt_t[:, o0 + s, m0 : m0 + md.m_tile]
                    )
            else:
                nc_.sync.dma_start(
                    out=t[:],
                    in_=xt_t[:, o0 : o0 + md.k_subtiles, m0 : m0 + md.m_tile],
                )
            return t

        kxm_shape = ShapeInfo(pdims=((P, ko),), fdims=(mc,))

        def kxn_producer(nc_, md):
            return w_sb[:, ts(md.k_tile_idx, md.k_subtiles), ts(md.n_tile_idx, md.n_tile)]

        kxn_shape = ShapeInfo(pdims=((P, ko),), fdims=(nc_dim,))

        out_t = out.ap().rearrange("(o p) n -> p o n", p=P)

        def add_bias_store_reducer(nc_, psum, sbuf, md):
            # psum -> sbuf with the bias added, then store this subtile
            # immediately (finer-grained than the stock whole-tile consumer,
            # so stores overlap the remaining evictions and the tail drains
            # faster).
            sz = md.n_subtile_slice_size
            nc_.vector.tensor_add(
                out=sbuf[:, :, :sz],
                in0=psum[:, :sz],
                in1=bias_sb[: psum.shape[0], md.n_subtile_slice],
            )
            po = md.m_tile_idx * md.m_subtiles + md.m_subtile_idx
            nc_.sync.dma_start(
                out=out_t[:, po : po + 1, md.n_subtile_slice], in_=sbuf[:, :, :sz]
            )

        for _ in range(reps):
            composable_matmul_tile_kernel(
                tc=tc,
                kxm_shape=kxm_shape,
                kxn_shape=kxn_shape,
                output_type=mybir.dt.float32,
                kxm_producer=kxm_producer,
                kxn_producer=kxn_producer,
                mxn_consumer=lambda nc_, tile_, md: None,
                mxn_subtile_reducer=add_bias_store_reducer,
                MAX_K_TILE_SIZE=MAX_K_TILE,
                psum_n_bufs=2,
            )

    nc.compile()
    return nc


_NC_CACHE = None


def _get_nc():
    global _NC_CACHE
    if _NC_CACHE is None:
        _NC_CACHE = build_nc()
    return _NC_CACHE


def kernel(x: np.ndarray, w: np.ndarray, b: np.ndarray) -> np.ndarray:
    from concourse.bass_utils import run_bass_kernel_spmd

    x = np.asarray(x, dtype=np.float32)
    w = np.asarray(w, dtype=np.float32)
    b = np.asarray(b, dtype=np.float32)

    bf16 = ml_dtypes.bfloat16
    x_bf = x.astype(bf16)
    w_bf = np.sign(w).astype(bf16)

    # Unique transposed shards (x^T per batch group, sign(w)^T per feature group).
    xt_shards = [
        np.ascontiguousarray(x_bf[bi * MC : (bi + 1) * MC, :].T) for bi in range(P_BATCH)
    ]
    wt_shards = [
        np.ascontiguousarray(w_bf[fi * NC : (fi + 1) * NC, :].T) for fi in range(P_FEAT)
    ]
    b_shards = [np.ascontiguousarray(b[fi * NC : (fi + 1) * NC]) for fi in range(P_FEAT)]

    in_maps = []
    for c in range(N_CORES):
        bi, fi = divmod(c, P_FEAT)
        in_maps.append(
            {"xt": xt_shards[bi], "wt": wt_shards[fi], "bias": b_shards[fi]}
        )

    nc = _get_nc()
    results = run_bass_kernel_spmd(nc, in_maps, core_ids=list(range(N_CORES))).results

    out = np.empty((M, N), dtype=np.float32)
    for c in range(N_CORES):
        bi, fi = divmod(c, P_FEAT)
        out[bi * MC : (bi + 1) * MC, fi * NC : (fi + 1) * NC] = results[c]["out"]
    return out
